# revision 1
# baseline (speedup 1.0000x reference)
"""AttentionBlock (B=8, C=512, N=2048, 8 heads) on 8 TRN2 NeuronCores.

Sharding: data-parallel over batch — one batch per core; all 8 heads of a
batch are computed on its core.

Per-core dataflow (d_head=64):
  - qkv projection as fp32r matmuls from SBUF-resident x [512,2048] and the
    host-reordered W^T [512,1536]; q/k land per head-pair in bf16 tiles
    [128,2048] (rows 0-63 head 2p, rows 64-127 head 2p+1), v lands
    transposed (vt[s, c]) in fp32r with a constant ones column appended.
  - per (pair, 512-wide t-block): S^T[s,t] = k^T q via two row-group-packed
    bf16 matmuls into one [128,1024] psum tile; exp(S/64) on ScalarE
    (psum -> fp32r SBUF); PV matmul o[65,512] += vt[s,65]^T @ exp-tile
    accumulates the attention output AND (via the ones column, row 64) the
    softmax denominator. PV is software-pipelined one step behind exp so the
    tensor engine never waits on ScalarE.
  - normalize: recip = 1/o[64] (DVE), broadcast across partitions (GPSIMD
    partition_broadcast), multiply o[0:64] (DVE), DMA out.
No max-subtraction is needed: logits = q.k/64 are ~N(0,1) here, far from
fp32 exp overflow.
"""

from contextlib import ExitStack

import numpy as np

import concourse.bacc as bacc
import concourse.bass_utils as bass_utils
import concourse.mybir as mybir
import concourse.tile as tile

F32 = mybir.dt.float32
F32R = mybir.dt.float32r
BF16 = mybir.dt.bfloat16
AF = mybir.ActivationFunctionType

B = 8
HEAD = 8
D = 64          # d_head
C = 512         # channels
N = 2048        # sequence
PAIRS = HEAD // 2
CT = C // 128   # contraction tiles for the projection
NT = N // 512   # 512-wide n/t blocks
ST = N // 128   # s-tiles
VW = 65         # vt slot width: 64 v cols + ones col


def _col_perm():
    """Column order for the host-side reordered W.T ([512, 1536]).

    Cols 0..1023: per pair p: k_{2p}, k_{2p+1}, q_{2p}, q_{2p+1} (64 each).
    Cols 1024..1535: v_0 .. v_7.  In the original qkv rows, head h uses
    q: h*192+0..64, k: h*192+64..128, v: h*192+128..192.
    """
    cols = []
    for p in range(PAIRS):
        h0, h1 = 2 * p, 2 * p + 1
        cols += list(range(h0 * 192 + 64, h0 * 192 + 128))
        cols += list(range(h1 * 192 + 64, h1 * 192 + 128))
        cols += list(range(h0 * 192, h0 * 192 + 64))
        cols += list(range(h1 * 192, h1 * 192 + 64))
    for h in range(HEAD):
        cols += list(range(h * 192 + 128, h * 192 + 192))
    return np.array(cols, dtype=np.int64)


def build(repeat: int = 1, pt_bufs: int = 6):
    """Build the per-core Bass kernel; repeat>1 wraps the body in a For_i
    loop over the same data (used only for wall-clock benchmarking)."""
    nc = bacc.Bacc("TRN2", target_bir_lowering=False, debug=False, num_devices=B)
    x_d = nc.dram_tensor("x", [C, N], F32, kind="ExternalInput").ap()
    wt_d = nc.dram_tensor("wt", [C, 3 * C], F32, kind="ExternalInput").ap()
    out_d = nc.dram_tensor("out", [C, N], F32, kind="ExternalOutput").ap()

    with tile.TileContext(nc) as tc:
        if repeat == 1:
            _emit(nc, tc, x_d, wt_d, out_d, pt_bufs)
        else:
            with tc.For_i(0, repeat, 1) as _i:
                _emit(nc, tc, x_d, wt_d, out_d, pt_bufs)
    nc.compile()
    return nc


def _emit(nc, tc, x_d, wt_d, out_d, pt_bufs):
    with ExitStack() as ctx:
        ctx.enter_context(nc.allow_low_precision(reason="bf16 attention weights"))
        persist = ctx.enter_context(tc.tile_pool(name="persist", bufs=1))
        qk_pool = ctx.enter_context(tc.tile_pool(name="qk", bufs=2 * PAIRS))
        pt_pool = ctx.enter_context(tc.tile_pool(name="pt", bufs=pt_bufs))
        sm_pool = ctx.enter_context(tc.tile_pool(name="small", bufs=8))
        ob_pool = ctx.enter_context(tc.tile_pool(name="ob", bufs=4))
        # proj tiles and PV output accumulators share one 4-slot pool (1 bank
        # per slot) so consecutive j-blocks can overlap their accumulators;
        # s_ps holds the S^T exp inputs ([128,1024] = 2 banks each).
        mix_ps = ctx.enter_context(tc.tile_pool(name="mix_ps", bufs=4, space="PSUM"))
        s_ps = ctx.enter_context(tc.tile_pool(name="s_ps", bufs=2, space="PSUM"))

        x_sb = persist.tile([128, CT, N], F32R, tag="x")
        wt_sb = persist.tile([128, CT, 3 * C], F32R, tag="wt")
        # loads are chunked in consumption order so the first projection
        # matmuls start ~4us in instead of waiting for the full 7MB
        def ld_wt(ct, c0, c1):
            nc.sync.dma_start(out=wt_sb[:, ct, c0:c1],
                              in_=wt_d[ct * 128:(ct + 1) * 128, c0:c1].bitcast(F32R))

        def ld_x(ct, c0, c1):
            nc.sync.dma_start(out=x_sb[:, ct, c0:c1],
                              in_=x_d[ct * 128:(ct + 1) * 128, c0:c1].bitcast(F32R))

        for ct in range(CT):
            ld_wt(ct, 0, 256)        # qk columns for pair 0
        for ct in range(CT):
            ld_x(ct, 0, 512)
        for ct in range(CT):
            ld_wt(ct, 1024, 1536)    # v columns
        for ct in range(CT):
            ld_x(ct, 512, 2048)
        for ct in range(CT):
            ld_wt(ct, 256, 1024)     # qk columns for pairs 1-3

        # vt ones column: memset of f32r is invalid ISA, so round-copy from
        # an f32 ones tile on the DVE (a sanctioned f32r rounding producer)
        vt_sb = persist.tile([128, ST, HEAD, VW], F32R, tag="vt")
        ones_sb = persist.tile([128, ST * HEAD], F32, tag="ones")
        nc.vector.memset(ones_sb, 1.0)
        nc.vector.tensor_copy(
            out=vt_sb[:, :, :, 64],
            in_=ones_sb.rearrange("p (s h) -> p s h", h=HEAD))

        def qkproj(p):
            tiles = []
            for blk in (p * 256, p * 256 + 128):  # k-block, q-block
                t = qk_pool.tile([128, N], BF16, tag="qk")
                for nt in range(NT):
                    ps = mix_ps.tile([128, 512], F32, tag="mix")
                    for ct in range(CT):
                        nc.tensor.matmul(
                            ps,
                            lhsT=wt_sb[:, ct, blk:blk + 128],
                            rhs=x_sb[:, ct, nt * 512:(nt + 1) * 512],
                            start=(ct == 0), stop=(ct == CT - 1),
                        )
                    nc.vector.tensor_copy(out=t[:, nt * 512:(nt + 1) * 512], in_=ps)
                tiles.append(t)
            return tiles

        def vproj():
            for i in range(ST):
                ps = mix_ps.tile([128, 512], F32, tag="mix")
                for ct in range(CT):
                    nc.tensor.matmul(
                        ps,
                        lhsT=x_sb[:, ct, i * 128:(i + 1) * 128],
                        rhs=wt_sb[:, ct, 1024:1536],
                        start=(ct == 0), stop=(ct == CT - 1),
                    )
                nc.vector.tensor_copy(
                    out=vt_sb[:, i, :, 0:64],
                    in_=ps.rearrange("p (h d) -> p h d", h=HEAD),
                )

        def phase_b(p, kt, qt):
            h0, h1 = 2 * p, 2 * p + 1
            for j in range(NT):
                o0 = mix_ps.tile([65, 512], F32, tag="mix")
                o1 = mix_ps.tile([65, 512], F32, tag="mix")
                pts = [None] * ST
                # software pipeline: PV trails S/exp by TWO steps so the PE
                # never comes near the ACT dependency (measured ~1.3x on the
                # composite i-step vs a 1-deep pipeline).
                for i in range(ST + 2):
                    if i < ST:
                        sp = s_ps.tile([128, 1024], F32, tag="sps")
                        nc.tensor.matmul(
                            sp[:, 0:512],
                            lhsT=kt[0:64, i * 128:(i + 1) * 128],
                            rhs=qt[0:64, j * 512:(j + 1) * 512],
                            start=True, stop=True,
                        )
                        nc.tensor.matmul(
                            sp[:, 512:1024],
                            lhsT=kt[64:128, i * 128:(i + 1) * 128],
                            rhs=qt[64:128, j * 512:(j + 1) * 512],
                            start=True, stop=True,
                        )
                        pt = pt_pool.tile([128, 1024], F32R, tag="pt")
                        nc.scalar.activation(out=pt, in_=sp, func=AF.Exp,
                                             scale=1.0 / D)
                        pts[i] = pt
                    if i > 1:
                        pt = pts[i - 2]
                        nc.tensor.matmul(
                            o0, lhsT=vt_sb[:, i - 2, h0, 0:65], rhs=pt[:, 0:512],
                            start=(i == 2), stop=(i == ST + 1),
                        )
                        nc.tensor.matmul(
                            o1, lhsT=vt_sb[:, i - 2, h1, 0:65],
                            rhs=pt[:, 512:1024],
                            start=(i == 2), stop=(i == ST + 1),
                        )
                for h, o in ((h0, o0), (h1, o1)):
                    recip = sm_pool.tile([1, 512], F32, tag="recip")
                    nc.vector.reciprocal(out=recip, in_=o[64:65, :])
                    bcast = sm_pool.tile([64, 512], F32, tag="bcast")
                    nc.gpsimd.partition_broadcast(bcast, recip)
                    ob = ob_pool.tile([64, 512], F32, tag="ob")
                    nc.vector.tensor_mul(ob, o[0:64, :], bcast)
                    nc.sync.dma_start(
                        out=out_d[h * D:(h + 1) * D, j * 512:(j + 1) * 512], in_=ob
                    )

        cur = qkproj(0)
        vproj()
        for p in range(PAIRS):
            phase_b(p, *cur)
            if p + 1 < PAIRS:
                cur = qkproj(p + 1)


_NC_CACHE = {}


def _get_nc(repeat=1):
    if repeat not in _NC_CACHE:
        _NC_CACHE[repeat] = build(repeat=repeat)
    return _NC_CACHE[repeat]


def kernel(x, W):
    """Full-input entry point: x [8,512,2048] f32, W [1536,512] f32 ->
    out [8,512,2048] f32. Shards batch over 8 cores internally."""
    x = np.asarray(x, dtype=np.float32)
    W = np.asarray(W, dtype=np.float32)
    assert x.shape == (B, C, N) and W.shape == (3 * C, C)
    nc = _get_nc()
    wt = np.ascontiguousarray(W.T[:, _col_perm()])
    in_maps = [{"x": np.ascontiguousarray(x[b]), "wt": wt} for b in range(B)]
    res = bass_utils.run_bass_kernel_spmd(nc, in_maps, core_ids=list(range(B)))
    return np.stack([res.results[b]["out"] for b in range(B)])

